# revision 13
# baseline (speedup 1.0000x reference)
"""MoE gate kernel for TRN2: logits = h @ W.T + bias; softmax; top-2; renorm.

Data-parallel over 8 NeuronCores: token dim B=16384 sharded to 2048/core,
weight (64, 4096) + bias replicated. Per core:
  - h loaded naturally [128 tok, 4096 d]; PE-transposes build hT in PSUM
    (fp32 has no DMA transpose on TRN2), two d-blocks per PSUM bank.
  - float32r matmuls (1 cycle/row vs 4 for fp32) with FULL fp32 accuracy
    via an error-compensated split: the stationary weight packs
    [w_r | w_err] (w_err = w - f32r(w)) into 128 columns, and the moving
    side runs twice - once with h_r = f32r(hT) and once with
    h_err = f32r(hT - h_r). All four product terms accumulate into one
    PSUM tile; rows 0:64 + rows 64:128 = (w_r+w_err)@(h_r+h_err) = w@h
    with residual ~2^-26.
  - The PE instruction stream interleaves the accumulating matmuls into
    the transpose stream with a 2/3-pair lag, so the PE stays busy while
    scalar round-copies (h_r) and vector subs (h_err) drain the PSUM
    transpose banks.
  - logits.T transposed back to [128 tok, 64 e]; vector max8/idx8 gives
    top-2; renormalized weights via w1 = 1/(1+exp(l2-l1)), w2 = e2*w1
    (exactly softmax-renorm restricted to the top 2).
"""
import numpy as np
import concourse.bacc as bacc
import concourse.mybir as mybir
from concourse.tile import TileContext
from concourse.bass_utils import run_bass_kernel_spmd
from concourse.masks import make_identity

N_CORES = 8
B = 16384
D = 4096
E = 64
B_SHARD = B // N_CORES      # 2048
CHUNK = 256
N_CHUNKS = B_SHARD // CHUNK  # 8
DBLK = D // 128              # 32
NPAIR = DBLK // 2            # 16 d-block pairs
TSUB = CHUNK // 128          # 2

F32 = mybir.dt.float32
F32R = mybir.dt.float32r
U32 = mybir.dt.uint32
I32 = mybir.dt.int32
AF = mybir.ActivationFunctionType


def _build():
    nc = bacc.Bacc("TRN2", target_bir_lowering=False, debug=False,
                   num_devices=N_CORES)
    h_d = nc.dram_tensor("h", [B_SHARD, D], F32, kind="ExternalInput")
    w_d = nc.dram_tensor("weight", [E, D], F32, kind="ExternalInput")
    b_d = nc.dram_tensor("bias", [E], F32, kind="ExternalInput")
    ow_d = nc.dram_tensor("topk_w", [B_SHARD, 2], F32, kind="ExternalOutput")
    oi_d = nc.dram_tensor("topk_idx", [B_SHARD, 2], I32, kind="ExternalOutput")

    with TileContext(nc) as tc:
        with (
            tc.tile_pool(name="const", bufs=1) as constp,
            tc.tile_pool(name="hnat", bufs=2) as hnatp,
            tc.tile_pool(name="htr", bufs=2) as htrp,
            tc.tile_pool(name="hte", bufs=1) as htep,
            tc.tile_pool(name="small", bufs=3) as smallp,
            tc.tile_pool(name="outb", bufs=2) as outbp,
            tc.tile_pool(name="tps", bufs=5, space="PSUM") as tpsp,
            tc.tile_pool(name="lps", bufs=2, space="PSUM") as lpsp,
            tc.tile_pool(name="ltps", bufs=1, space="PSUM") as ltpsp,
        ):
            ident = constp.tile([128, 128], F32, name="ident")
            make_identity(nc, ident[:])
            bias_sb = constp.tile([E, 1], F32, name="bias_sb")
            nc.sync.dma_start(out=bias_sb[:],
                              in_=b_d.ap().rearrange("(e o) -> e o", o=1))
            wnat = constp.tile([E, D], F32, name="wnat")
            nc.sync.dma_start(out=wnat[:], in_=w_d[:])
            # wt slot d: [w_r (64 cols) | w_err (64 cols)], both f32r
            wt = constp.tile([128, DBLK * 128], F32R, name="wt")
            # staging for W.T blocks: 4 tiles x 8 d-blocks (8*64 = 512 cols)
            wps = [None] * 4

            def emit_w_transposes(g):  # g in 0..3, 8 d-blocks each
                wps[g] = tpsp.tile([128, 512], F32, name=f"wps_{g}", tag="tp")
                for j in range(8):
                    d = 8 * g + j
                    nc.tensor.transpose(wps[g][:, 64 * j:64 * j + E],
                                        wnat[:, 128 * d:128 * (d + 1)],
                                        ident[0:E, 0:E])

            def emit_w_drain(g):
                # strided copy: wps[g][:, 64j:64j+64] -> wt[:, 128(8g+j):+64]
                src = wps[g][:].rearrange("p (j e) -> p j e", j=8)
                dst = wt[:, 512 * g * 2:512 * (g * 2 + 2)].rearrange(
                    "p (j two e) -> p j (two e)", j=8, two=2)[:, :, 0:E]
                nc.scalar.copy(dst, src)
                nc.vector.tensor_sub(
                    wt[:, 512 * g * 2:512 * (g * 2 + 2)].rearrange(
                        "p (j two e) -> p j (two e)", j=8, two=2)[:, :, E:128],
                    src,
                    dst.bitcast(F32))

            # --- main loop over token chunks ---
            finalize = None
            for c in range(N_CHUNKS):
                hn = [hnatp.tile([128, D], F32, name=f"hn_{c}_{s}",
                                 tag=f"hn_{s}") for s in range(TSUB)]
                for s in range(TSUB):
                    t0 = c * CHUNK + s * 128
                    nc.sync.dma_start(out=hn[s][:], in_=h_d[t0:t0 + 128, :])

                htr = [htrp.tile([128, 2 * CHUNK], F32R, name=f"htr_{c}_{q}",
                                 tag=f"htr_{q}") for q in range(NPAIR)]
                hte = [htep.tile([128, 2 * CHUNK], F32R, name=f"hte_{c}_{q}",
                                 tag=f"hte_{q}") for q in range(NPAIR)]
                lp = lpsp.tile([128, CHUNK], F32, name=f"lp_{c}", tag="lp")
                n_mm = 0  # matmuls emitted so far (A then B per index)

                def emit_mm_a(d):
                    nonlocal n_mm
                    q, k = divmod(d, 2)
                    nc.tensor.matmul(lp[:], wt[:, 128 * d:128 * (d + 1)],
                                     htr[q][:, 256 * k:256 * (k + 1)],
                                     start=(n_mm == 0), stop=False)
                    n_mm += 1

                def emit_mm_b(d):
                    nonlocal n_mm
                    q, k = divmod(d, 2)
                    nc.tensor.matmul(lp[:], wt[:, 128 * d:128 * (d + 1)],
                                     hte[q][:, 256 * k:256 * (k + 1)],
                                     start=False, stop=(n_mm == 2 * DBLK - 1))
                    n_mm += 1

                for q in range(NPAIR):
                    tp = tpsp.tile([128, 2 * CHUNK], F32, name=f"tp_{c}_{q}",
                                   tag="tp")
                    for k in range(2):
                        d = 2 * q + k
                        for s in range(TSUB):
                            nc.tensor.transpose(
                                tp[:, 256 * k + 128 * s:
                                   256 * k + 128 * (s + 1)],
                                hn[s][:, 128 * d:128 * (d + 1)], ident[:])
                    nc.scalar.copy(htr[q][:], tp[:])
                    nc.vector.tensor_sub(hte[q][:], tp[:],
                                         htr[q][:].bitcast(F32))

                    # previous chunk's epilogue woven into this transpose
                    # stream so its PE ops fill the copy/sub drain stalls
                    if q == 3 and finalize is not None:
                        finalize()
                        finalize = None

                    if c == 0:
                        # W setup woven into chunk 0's transpose stream
                        if 1 <= q <= 4:
                            emit_w_transposes(q - 1)
                            if q % 2 == 0:
                                emit_w_drain(q // 2 - 1)
                        elif q == 5:
                            emit_w_drain(2)
                        elif q == 6:
                            emit_w_drain(3)
                        # defer MMs until W is drained
                        if q >= 8:
                            emit_mm_a(2 * (q - 8))
                            emit_mm_a(2 * (q - 8) + 1)
                        if q >= 9:
                            emit_mm_b(2 * (q - 9))
                            emit_mm_b(2 * (q - 9) + 1)
                    else:
                        if q >= 2:
                            emit_mm_a(2 * (q - 2))
                            emit_mm_a(2 * (q - 2) + 1)
                        if q >= 3:
                            emit_mm_b(2 * (q - 3))
                            emit_mm_b(2 * (q - 3) + 1)

                lag_a, lag_b = (8, 9) if c == 0 else (2, 3)
                for q in range(NPAIR - lag_a, NPAIR):
                    emit_mm_a(2 * q)
                    emit_mm_a(2 * q + 1)
                for q in range(NPAIR - lag_b, NPAIR):
                    emit_mm_b(2 * q)
                    emit_mm_b(2 * q + 1)

                def finalize(c=c, lp=lp):
                    lsb0 = smallp.tile([E, CHUNK], F32, name=f"lsb0_{c}",
                                       tag="lsb0")
                    nc.scalar.activation(lsb0[:], lp[0:E, :], AF.Identity,
                                         bias=bias_sb[:])
                    lsb = smallp.tile([E, CHUNK], F32, name=f"lsb_{c}",
                                      tag="lsb")
                    nc.vector.tensor_add(lsb[:], lp[E:128, :], lsb0[:])

                    # back to [128 tok, 64 e]; top-2; renorm
                    ow = outbp.tile([128, 2 * TSUB], F32, name=f"ow_{c}",
                                    tag="ow")
                    oi = outbp.tile([128, 2 * TSUB], I32, name=f"oi_{c}",
                                    tag="oi")
                    for s in range(TSUB):
                        ltp = ltpsp.tile([128, E], F32, name=f"ltp_{c}_{s}",
                                         tag="ltp")
                        nc.tensor.transpose(ltp[:],
                                            lsb[:, 128 * s:128 * (s + 1)],
                                            ident[0:E, 0:E])
                        lgt = smallp.tile([128, E], F32, name=f"lgt_{c}_{s}",
                                          tag="lgt")
                        nc.scalar.copy(lgt[:], ltp[:])
                        m8 = smallp.tile([128, 8], F32, name=f"m8_{c}_{s}",
                                         tag="m8")
                        i8 = smallp.tile([128, 8], U32, name=f"i8_{c}_{s}",
                                         tag="i8")
                        nc.vector.max_with_indices(m8[:], i8[:], lgt[:])

                        dd = smallp.tile([128, 1], F32, name=f"dd_{c}_{s}",
                                         tag="dd")
                        e2 = smallp.tile([128, 1], F32, name=f"e2_{c}_{s}",
                                         tag="e2")
                        den = smallp.tile([128, 1], F32, name=f"den_{c}_{s}",
                                          tag="den")
                        nc.vector.tensor_sub(dd[:], m8[:, 1:2], m8[:, 0:1])
                        nc.scalar.activation(e2[:], dd[:], AF.Exp)
                        nc.vector.tensor_scalar(den[:], e2[:], 1.0,
                                                scalar2=None,
                                                op0=mybir.AluOpType.add)
                        nc.vector.reciprocal(ow[:, 2 * s:2 * s + 1], den[:])
                        nc.vector.tensor_mul(ow[:, 2 * s + 1:2 * s + 2],
                                             e2[:], ow[:, 2 * s:2 * s + 1])
                        nc.vector.tensor_copy(oi[:, 2 * s:2 * s + 2],
                                              i8[:, 0:2].bitcast(I32))

                    t0 = c * CHUNK
                    nc.sync.dma_start(
                        out=ow_d[t0:t0 + CHUNK, :].rearrange(
                            "(s p) j -> p s j", s=TSUB),
                        in_=ow[:])
                    nc.sync.dma_start(
                        out=oi_d[t0:t0 + CHUNK, :].rearrange(
                            "(s p) j -> p s j", s=TSUB),
                        in_=oi[:])

            finalize()

    nc.compile()
    return nc


_NC = None


def _get_nc():
    global _NC
    if _NC is None:
        _NC = _build()
    return _NC


def run(h, weight, bias, trace=False):
    nc = _get_nc()
    h = np.ascontiguousarray(h, dtype=np.float32)
    weight = np.ascontiguousarray(weight, dtype=np.float32)
    bias = np.ascontiguousarray(bias, dtype=np.float32)
    in_maps = [{"h": h[i * B_SHARD:(i + 1) * B_SHARD], "weight": weight,
                "bias": bias} for i in range(N_CORES)]
    res = run_bass_kernel_spmd(nc, in_maps, list(range(N_CORES)), trace=trace)
    tw = np.concatenate([res.results[i]["topk_w"] for i in range(N_CORES)], 0)
    ti = np.concatenate([res.results[i]["topk_idx"] for i in range(N_CORES)], 0)
    return (tw.astype(np.float32), ti.astype(np.int32)), res


def kernel(h, weight, bias):
    (tw, ti), _ = run(h, weight, bias)
    return tw, ti


# revision 16
# speedup vs baseline: 1.2085x; 1.2085x over previous
"""MoE gate kernel for TRN2: logits = h @ W.T + bias; softmax; top-2; renorm.

Data-parallel over 8 NeuronCores: token dim B=16384 sharded to 2048/core,
weight (64, 4096) + bias replicated. Per core:
  - h loaded naturally [128 tok, 4096 d]; PE-transposes build hT in PSUM
    (fp32 has no DMA transpose on TRN2), two d-blocks per PSUM bank.
  - float32r matmuls (1 cycle/row vs 4 for fp32) with FULL fp32 accuracy
    via an error-compensated split: the stationary weight packs
    [w_r | w_err] (w_err = w - f32r(w)) into 128 columns, and the moving
    side runs twice - once with h_r = f32r(hT) and once with
    h_err = f32r(hT - h_r). All four product terms accumulate into one
    PSUM tile; rows 0:64 + rows 64:128 = (w_r+w_err)@(h_r+h_err) = w@h
    with residual ~2^-26.
  - The PE instruction stream interleaves the accumulating matmuls into
    the transpose stream with a 2/3-pair lag, so the PE stays busy while
    scalar round-copies (h_r) and vector subs (h_err) drain the PSUM
    transpose banks.
  - logits.T transposed back to [128 tok, 64 e]; vector max8/idx8 gives
    top-2; renormalized weights via w1 = 1/(1+exp(l2-l1)), w2 = e2*w1
    (exactly softmax-renorm restricted to the top 2).
"""
import numpy as np
import concourse.bacc as bacc
import concourse.mybir as mybir
from concourse.tile import TileContext
from concourse.bass_utils import run_bass_kernel_spmd
from concourse.masks import make_identity

N_CORES = 8
B = 16384
D = 4096
E = 64
B_SHARD = B // N_CORES      # 2048
CHUNK = 256
N_CHUNKS = B_SHARD // CHUNK  # 8
DBLK = D // 128              # 32
NPAIR = DBLK // 2            # 16 d-block pairs
TSUB = CHUNK // 128          # 2

F32 = mybir.dt.float32
F32R = mybir.dt.float32r
U32 = mybir.dt.uint32
I32 = mybir.dt.int32
AF = mybir.ActivationFunctionType


def _build():
    nc = bacc.Bacc("TRN2", target_bir_lowering=False, debug=False,
                   num_devices=N_CORES)
    h_d = nc.dram_tensor("h", [B_SHARD, D], F32, kind="ExternalInput")
    w_d = nc.dram_tensor("weight", [E, D], F32, kind="ExternalInput")
    b_d = nc.dram_tensor("bias", [E], F32, kind="ExternalInput")
    ow_d = nc.dram_tensor("topk_w", [B_SHARD, 2], F32, kind="ExternalOutput")
    oi_d = nc.dram_tensor("topk_idx", [B_SHARD, 2], I32, kind="ExternalOutput")

    with TileContext(nc) as tc:
        with (
            tc.tile_pool(name="const", bufs=1) as constp,
            tc.tile_pool(name="hnat", bufs=2) as hnatp,
            tc.tile_pool(name="htr", bufs=2) as htrp,
            tc.tile_pool(name="hte", bufs=1) as htep,
            tc.tile_pool(name="small", bufs=3) as smallp,
            tc.tile_pool(name="outb", bufs=2) as outbp,
            tc.tile_pool(name="tps", bufs=4, space="PSUM") as tpsp,
            tc.tile_pool(name="lps", bufs=2, space="PSUM") as lpsp,
            tc.tile_pool(name="ltps", bufs=2, space="PSUM") as ltpsp,
        ):
            ident = constp.tile([128, 128], F32, name="ident")
            make_identity(nc, ident[:])
            bias_sb = constp.tile([E, 1], F32, name="bias_sb")
            nc.sync.dma_start(out=bias_sb[:],
                              in_=b_d.ap().rearrange("(e o) -> e o", o=1))
            wnat = constp.tile([E, D], F32, name="wnat")
            nc.sync.dma_start(out=wnat[:], in_=w_d[:])
            # wt slot d: [w_r (64 cols) | w_err (64 cols)], both f32r
            wt = constp.tile([128, DBLK * 128], F32R, name="wt")
            # staging for W.T blocks: 4 tiles x 8 d-blocks (8*64 = 512 cols)
            wps = [None] * 4

            def emit_w_transposes(g):  # g in 0..3, 8 d-blocks each
                wps[g] = tpsp.tile([128, 512], F32, name=f"wps_{g}", tag="tp")
                for j in range(8):
                    d = 8 * g + j
                    nc.tensor.transpose(wps[g][:, 64 * j:64 * j + E],
                                        wnat[:, 128 * d:128 * (d + 1)],
                                        ident[0:E, 0:E])

            def emit_w_drain(g):
                # strided copy: wps[g][:, 64j:64j+64] -> wt[:, 128(8g+j):+64]
                src = wps[g][:].rearrange("p (j e) -> p j e", j=8)
                dst = wt[:, 512 * g * 2:512 * (g * 2 + 2)].rearrange(
                    "p (j two e) -> p j (two e)", j=8, two=2)[:, :, 0:E]
                nc.scalar.copy(dst, src)
                nc.vector.tensor_sub(
                    wt[:, 512 * g * 2:512 * (g * 2 + 2)].rearrange(
                        "p (j two e) -> p j (two e)", j=8, two=2)[:, :, E:128],
                    src,
                    dst.bitcast(F32))

            # --- main loop over token chunks ---
            finalize = None
            for c in range(N_CHUNKS):
                hn = [hnatp.tile([128, D], F32, name=f"hn_{c}_{s}",
                                 tag=f"hn_{s}") for s in range(TSUB)]
                for s in range(TSUB):
                    t0 = c * CHUNK + s * 128
                    nc.sync.dma_start(out=hn[s][:], in_=h_d[t0:t0 + 128, :])

                htr = [htrp.tile([128, 2 * CHUNK], F32R, name=f"htr_{c}_{q}",
                                 tag=f"htr_{q}") for q in range(NPAIR)]
                hte = [htep.tile([128, 2 * CHUNK], F32R, name=f"hte_{c}_{q}",
                                 tag=f"hte_{q}") for q in range(NPAIR)]
                lp = lpsp.tile([128, CHUNK], F32, name=f"lp_{c}", tag="lp")
                n_mm = 0  # matmuls emitted so far (A then B per index)

                def emit_mm_a(d):
                    nonlocal n_mm
                    q, k = divmod(d, 2)
                    nc.tensor.matmul(lp[:], wt[:, 128 * d:128 * (d + 1)],
                                     htr[q][:, 256 * k:256 * (k + 1)],
                                     start=(n_mm == 0), stop=False)
                    n_mm += 1

                def emit_mm_b(d):
                    nonlocal n_mm
                    q, k = divmod(d, 2)
                    nc.tensor.matmul(lp[:], wt[:, 128 * d:128 * (d + 1)],
                                     hte[q][:, 256 * k:256 * (k + 1)],
                                     start=False, stop=(n_mm == 2 * DBLK - 1))
                    n_mm += 1

                for q in range(NPAIR):
                    tp = tpsp.tile([128, 2 * CHUNK], F32, name=f"tp_{c}_{q}",
                                   tag="tp")
                    for k in range(2):
                        d = 2 * q + k
                        for s in range(TSUB):
                            nc.tensor.transpose(
                                tp[:, 256 * k + 128 * s:
                                   256 * k + 128 * (s + 1)],
                                hn[s][:, 128 * d:128 * (d + 1)], ident[:])
                    nc.scalar.copy(htr[q][:], tp[:])
                    nc.vector.tensor_sub(hte[q][:], tp[:],
                                         htr[q][:].bitcast(F32))

                    # previous chunk's topk (PE transpose-back + vector ops)
                    # woven into this transpose stream; its scalar/vector
                    # logit-combine already ran at the chunk boundary
                    if q == 2 and finalize is not None:
                        finalize()
                        finalize = None

                    if c == 0:
                        # W setup woven into chunk 0's transpose stream
                        if 1 <= q <= 4:
                            emit_w_transposes(q - 1)
                            if q % 2 == 0:
                                emit_w_drain(q // 2 - 1)
                        elif q == 5:
                            emit_w_drain(2)
                        elif q == 6:
                            emit_w_drain(3)
                        # defer MMs until W is drained
                        if q >= 8:
                            emit_mm_a(2 * (q - 8))
                            emit_mm_a(2 * (q - 8) + 1)
                        if q >= 9:
                            emit_mm_b(2 * (q - 9))
                            emit_mm_b(2 * (q - 9) + 1)
                    else:
                        if q >= 2:
                            emit_mm_a(2 * (q - 2))
                            emit_mm_a(2 * (q - 2) + 1)
                        if q >= 3:
                            emit_mm_b(2 * (q - 3))
                            emit_mm_b(2 * (q - 3) + 1)

                lag_a, lag_b = (8, 9) if c == 0 else (2, 3)
                for q in range(NPAIR - lag_a, NPAIR):
                    emit_mm_a(2 * q)
                    emit_mm_a(2 * q + 1)
                for q in range(NPAIR - lag_b, NPAIR):
                    emit_mm_b(2 * q)
                    emit_mm_b(2 * q + 1)

                # logit combine right after the stop-matmul: scalar/vector
                # are idle at the chunk boundary
                lsb0 = smallp.tile([E, CHUNK], F32, name=f"lsb0_{c}",
                                   tag="lsb0")
                nc.scalar.activation(lsb0[:], lp[0:E, :], AF.Identity,
                                     bias=bias_sb[:])
                lsb = smallp.tile([E, CHUNK], F32, name=f"lsb_{c}",
                                  tag="lsb")
                nc.vector.tensor_add(lsb[:], lp[E:128, :], lsb0[:])

                def finalize(c=c, lsb=lsb):
                    # back to [128 tok, 64 e]; top-2; renorm
                    ow = outbp.tile([128, 2 * TSUB], F32, name=f"ow_{c}",
                                    tag="ow")
                    oi = outbp.tile([128, 2 * TSUB], I32, name=f"oi_{c}",
                                    tag="oi")
                    for s in range(TSUB):
                        ltp = ltpsp.tile([128, E], F32, name=f"ltp_{c}_{s}",
                                         tag="ltp")
                        nc.tensor.transpose(ltp[:],
                                            lsb[:, 128 * s:128 * (s + 1)],
                                            ident[0:E, 0:E])
                        lgt = smallp.tile([128, E], F32, name=f"lgt_{c}_{s}",
                                          tag="lgt")
                        nc.scalar.copy(lgt[:], ltp[:])
                        m8 = smallp.tile([128, 8], F32, name=f"m8_{c}_{s}",
                                         tag="m8")
                        i8 = smallp.tile([128, 8], U32, name=f"i8_{c}_{s}",
                                         tag="i8")
                        nc.vector.max_with_indices(m8[:], i8[:], lgt[:])

                        dd = smallp.tile([128, 1], F32, name=f"dd_{c}_{s}",
                                         tag="dd")
                        e2 = smallp.tile([128, 1], F32, name=f"e2_{c}_{s}",
                                         tag="e2")
                        den = smallp.tile([128, 1], F32, name=f"den_{c}_{s}",
                                          tag="den")
                        nc.vector.tensor_sub(dd[:], m8[:, 1:2], m8[:, 0:1])
                        nc.scalar.activation(e2[:], dd[:], AF.Exp)
                        nc.vector.tensor_scalar(den[:], e2[:], 1.0,
                                                scalar2=None,
                                                op0=mybir.AluOpType.add)
                        nc.vector.reciprocal(ow[:, 2 * s:2 * s + 1], den[:])
                        nc.vector.tensor_mul(ow[:, 2 * s + 1:2 * s + 2],
                                             e2[:], ow[:, 2 * s:2 * s + 1])
                        nc.vector.tensor_copy(oi[:, 2 * s:2 * s + 2],
                                              i8[:, 0:2].bitcast(I32))

                    t0 = c * CHUNK
                    nc.sync.dma_start(
                        out=ow_d[t0:t0 + CHUNK, :].rearrange(
                            "(s p) j -> p s j", s=TSUB),
                        in_=ow[:])
                    nc.sync.dma_start(
                        out=oi_d[t0:t0 + CHUNK, :].rearrange(
                            "(s p) j -> p s j", s=TSUB),
                        in_=oi[:])

            finalize()

    nc.compile()
    return nc


_NC = None


def _get_nc():
    global _NC
    if _NC is None:
        _NC = _build()
    return _NC


def run(h, weight, bias, trace=False):
    nc = _get_nc()
    h = np.ascontiguousarray(h, dtype=np.float32)
    weight = np.ascontiguousarray(weight, dtype=np.float32)
    bias = np.ascontiguousarray(bias, dtype=np.float32)
    in_maps = [{"h": h[i * B_SHARD:(i + 1) * B_SHARD], "weight": weight,
                "bias": bias} for i in range(N_CORES)]
    res = run_bass_kernel_spmd(nc, in_maps, list(range(N_CORES)), trace=trace)
    tw = np.concatenate([res.results[i]["topk_w"] for i in range(N_CORES)], 0)
    ti = np.concatenate([res.results[i]["topk_idx"] for i in range(N_CORES)], 0)
    return (tw.astype(np.float32), ti.astype(np.int32)), res


def kernel(h, weight, bias):
    (tw, ti), _ = run(h, weight, bias)
    return tw, ti


# revision 20
# speedup vs baseline: 1.2917x; 1.0688x over previous
"""MoE gate kernel for TRN2: logits = h @ W.T + bias; softmax; top-2; renorm.

Data-parallel over 8 NeuronCores: token dim B=16384 sharded to 2048/core,
weight (64, 4096) + bias replicated. Per core:
  - h loaded naturally [128 tok, 4096 d]; PE-transposes build hT in PSUM
    (fp32 has no DMA transpose on TRN2), two d-blocks per PSUM bank.
  - float32r matmuls (1 cycle/row vs 4 for fp32) with FULL fp32 accuracy
    via an error-compensated split: the stationary weight packs
    [w_r | w_err] (w_err = w - f32r(w)) into 128 columns, and the moving
    side runs twice - once with h_r = f32r(hT) and once with
    h_err = f32r(hT - h_r). All four product terms accumulate into one
    PSUM tile; rows 0:64 + rows 64:128 = (w_r+w_err)@(h_r+h_err) = w@h
    with residual ~2^-26.
  - The PE instruction stream interleaves the accumulating matmuls into
    the transpose stream with a 2/3-pair lag, so the PE stays busy while
    scalar round-copies (h_r) and vector subs (h_err) drain the PSUM
    transpose banks.
  - logits.T transposed back to [128 tok, 64 e]; vector max8/idx8 gives
    top-2; renormalized weights via w1 = 1/(1+exp(l2-l1)), w2 = e2*w1
    (exactly softmax-renorm restricted to the top 2).
"""
import numpy as np
import concourse.bacc as bacc
import concourse.mybir as mybir
from concourse.tile import TileContext
from concourse.bass_utils import run_bass_kernel_spmd
from concourse.masks import make_identity

N_CORES = 8
B = 16384
D = 4096
E = 64
B_SHARD = B // N_CORES      # 2048
CHUNK = 256
N_CHUNKS = B_SHARD // CHUNK  # 8
DBLK = D // 128              # 32
NPAIR = DBLK // 2            # 16 d-block pairs
TSUB = CHUNK // 128          # 2

F32 = mybir.dt.float32
F32R = mybir.dt.float32r
U32 = mybir.dt.uint32
I32 = mybir.dt.int32
AF = mybir.ActivationFunctionType


def _build():
    nc = bacc.Bacc("TRN2", target_bir_lowering=False, debug=False,
                   num_devices=N_CORES)
    h_d = nc.dram_tensor("h", [B_SHARD, D], F32, kind="ExternalInput")
    w_d = nc.dram_tensor("weight", [E, D], F32, kind="ExternalInput")
    b_d = nc.dram_tensor("bias", [E], F32, kind="ExternalInput")
    ow_d = nc.dram_tensor("topk_w", [B_SHARD, 2], F32, kind="ExternalOutput")
    oi_d = nc.dram_tensor("topk_idx", [B_SHARD, 2], I32, kind="ExternalOutput")

    with TileContext(nc) as tc:
        with (
            tc.tile_pool(name="const", bufs=1) as constp,
            tc.tile_pool(name="hnat", bufs=2) as hnatp,
            tc.tile_pool(name="htr", bufs=2) as htrp,
            tc.tile_pool(name="hte", bufs=1) as htep,
            tc.tile_pool(name="small", bufs=3) as smallp,
            tc.tile_pool(name="outb", bufs=2) as outbp,
            tc.tile_pool(name="tps", bufs=4, space="PSUM") as tpsp,
            tc.tile_pool(name="lps", bufs=2, space="PSUM") as lpsp,
            tc.tile_pool(name="ltps", bufs=2, space="PSUM") as ltpsp,
        ):
            ident = constp.tile([128, 128], F32, name="ident")
            make_identity(nc, ident[:])
            bias_sb = constp.tile([E, 1], F32, name="bias_sb")
            wnat = constp.tile([E, D], F32, name="wnat")
            # wt slot d: [w_r (64 cols) | w_err (64 cols)], both f32r
            wt = constp.tile([128, DBLK * 128], F32R, name="wt")
            # staging for W.T blocks: 4 tiles x 8 d-blocks (8*64 = 512 cols)
            wps = [None] * 4

            def emit_w_transposes(g):  # g in 0..3, 8 d-blocks each
                wps[g] = tpsp.tile([128, 512], F32, name=f"wps_{g}", tag="tp")
                for j in range(8):
                    d = 8 * g + j
                    nc.tensor.transpose(wps[g][:, 64 * j:64 * j + E],
                                        wnat[:, 128 * d:128 * (d + 1)],
                                        ident[0:E, 0:E])

            def emit_w_drain(g):
                # strided copy: wps[g][:, 64j:64j+64] -> wt[:, 128(8g+j):+64]
                src = wps[g][:].rearrange("p (j e) -> p j e", j=8)
                dst = wt[:, 512 * g * 2:512 * (g * 2 + 2)].rearrange(
                    "p (j two e) -> p j (two e)", j=8, two=2)[:, :, 0:E]
                nc.scalar.copy(dst, src)
                nc.vector.tensor_sub(
                    wt[:, 512 * g * 2:512 * (g * 2 + 2)].rearrange(
                        "p (j two e) -> p j (two e)", j=8, two=2)[:, :, E:128],
                    src,
                    dst.bitcast(F32))

            # --- main loop over token chunks ---
            # h loads as 1 MB halves, issued one chunk ahead so the SP
            # queue's output-DMA waits never delay the next chunk's loads
            hn_tiles = {}

            def emit_hn(c):
                hn = [[hnatp.tile([128, D // 2], F32, name=f"hn_{c}_{s}_{hh}",
                                  tag=f"hn_{s}_{hh}") for hh in range(2)]
                      for s in range(TSUB)]
                hn_tiles[c] = hn
                for hh in range(2):
                    for s in range(TSUB):
                        t0 = c * CHUNK + s * 128
                        nc.sync.dma_start(
                            out=hn[s][hh][:],
                            in_=h_d[t0:t0 + 128,
                                    2048 * hh:2048 * (hh + 1)])
                    if c == 0 and hh == 0:
                        nc.sync.dma_start(out=wnat[:], in_=w_d[:])
                        nc.sync.dma_start(
                            out=bias_sb[:],
                            in_=b_d.ap().rearrange("(e o) -> e o", o=1))

            finalize = None
            for c in range(N_CHUNKS):
                if c == 0:
                    emit_hn(0)
                    emit_hn(1)
                elif c + 1 < N_CHUNKS:
                    emit_hn(c + 1)
                hn = hn_tiles.pop(c)

                htr = [htrp.tile([128, 2 * CHUNK], F32R, name=f"htr_{c}_{q}",
                                 tag=f"htr_{q}") for q in range(NPAIR)]
                hte = [htep.tile([128, 2 * CHUNK], F32R, name=f"hte_{c}_{q}",
                                 tag=f"hte_{q}") for q in range(NPAIR)]
                lp = lpsp.tile([128, CHUNK], F32, name=f"lp_{c}", tag="lp")
                n_mm = 0  # matmuls emitted so far (A then B per index)

                def emit_mm_a(d):
                    nonlocal n_mm
                    q, k = divmod(d, 2)
                    nc.tensor.matmul(lp[:], wt[:, 128 * d:128 * (d + 1)],
                                     htr[q][:, 256 * k:256 * (k + 1)],
                                     start=(n_mm == 0), stop=False)
                    n_mm += 1

                def emit_mm_b(d):
                    nonlocal n_mm
                    q, k = divmod(d, 2)
                    nc.tensor.matmul(lp[:], wt[:, 128 * d:128 * (d + 1)],
                                     hte[q][:, 256 * k:256 * (k + 1)],
                                     start=False, stop=(n_mm == 2 * DBLK - 1))
                    n_mm += 1

                for q in range(NPAIR):
                    tp = tpsp.tile([128, 2 * CHUNK], F32, name=f"tp_{c}_{q}",
                                   tag="tp")
                    for k in range(2):
                        d = 2 * q + k
                        for s in range(TSUB):
                            dk = d % 16
                            nc.tensor.transpose(
                                tp[:, 256 * k + 128 * s:
                                   256 * k + 128 * (s + 1)],
                                hn[s][d // 16][:, 128 * dk:128 * (dk + 1)],
                                ident[:])
                    nc.scalar.copy(htr[q][:], tp[:])
                    nc.vector.tensor_sub(hte[q][:], tp[:],
                                         htr[q][:].bitcast(F32))

                    # previous chunk's topk (PE transpose-back + vector ops)
                    # woven into this transpose stream; its scalar/vector
                    # logit-combine already ran at the chunk boundary
                    if q == 2 and finalize is not None:
                        finalize()
                        finalize = None

                    if c == 0:
                        # W setup woven into chunk 0's transpose stream
                        # (wnat DMA lands after the first two h halves)
                        if 3 <= q <= 6:
                            emit_w_transposes(q - 3)
                        if 4 <= q <= 7:
                            emit_w_drain(q - 4)
                        # defer MMs until W is drained
                        if q >= 10:
                            emit_mm_a(2 * (q - 10))
                            emit_mm_a(2 * (q - 10) + 1)
                        if q >= 11:
                            emit_mm_b(2 * (q - 11))
                            emit_mm_b(2 * (q - 11) + 1)
                    else:
                        if q >= 2:
                            emit_mm_a(2 * (q - 2))
                            emit_mm_a(2 * (q - 2) + 1)
                        if q >= 3:
                            emit_mm_b(2 * (q - 3))
                            emit_mm_b(2 * (q - 3) + 1)

                lag_a, lag_b = (10, 11) if c == 0 else (2, 3)
                for q in range(NPAIR - lag_a, NPAIR):
                    emit_mm_a(2 * q)
                    emit_mm_a(2 * q + 1)
                for q in range(NPAIR - lag_b, NPAIR):
                    emit_mm_b(2 * q)
                    emit_mm_b(2 * q + 1)

                # logit combine right after the stop-matmul: scalar/vector
                # are idle at the chunk boundary
                lsb0 = smallp.tile([E, CHUNK], F32, name=f"lsb0_{c}",
                                   tag="lsb0")
                nc.scalar.activation(lsb0[:], lp[0:E, :], AF.Identity,
                                     bias=bias_sb[:])
                lsb = smallp.tile([E, CHUNK], F32, name=f"lsb_{c}",
                                  tag="lsb")
                nc.vector.tensor_add(lsb[:], lp[E:128, :], lsb0[:])

                def finalize(c=c, lsb=lsb):
                    # back to [128 tok, 64 e]; top-2; renorm
                    ow = outbp.tile([128, 2 * TSUB], F32, name=f"ow_{c}",
                                    tag="ow")
                    oi = outbp.tile([128, 2 * TSUB], I32, name=f"oi_{c}",
                                    tag="oi")
                    for s in range(TSUB):
                        ltp = ltpsp.tile([128, E], F32, name=f"ltp_{c}_{s}",
                                         tag="ltp")
                        nc.tensor.transpose(ltp[:],
                                            lsb[:, 128 * s:128 * (s + 1)],
                                            ident[0:E, 0:E])
                        lgt = smallp.tile([128, E], F32, name=f"lgt_{c}_{s}",
                                          tag="lgt")
                        nc.scalar.copy(lgt[:], ltp[:])
                        m8 = smallp.tile([128, 8], F32, name=f"m8_{c}_{s}",
                                         tag="m8")
                        i8 = smallp.tile([128, 8], U32, name=f"i8_{c}_{s}",
                                         tag="i8")
                        nc.vector.max_with_indices(m8[:], i8[:], lgt[:])

                        dd = smallp.tile([128, 1], F32, name=f"dd_{c}_{s}",
                                         tag="dd")
                        e2 = smallp.tile([128, 1], F32, name=f"e2_{c}_{s}",
                                         tag="e2")
                        den = smallp.tile([128, 1], F32, name=f"den_{c}_{s}",
                                          tag="den")
                        nc.vector.tensor_sub(dd[:], m8[:, 1:2], m8[:, 0:1])
                        nc.scalar.activation(e2[:], dd[:], AF.Exp)
                        nc.vector.tensor_scalar(den[:], e2[:], 1.0,
                                                scalar2=None,
                                                op0=mybir.AluOpType.add)
                        nc.vector.reciprocal(ow[:, 2 * s:2 * s + 1], den[:])
                        nc.vector.tensor_mul(ow[:, 2 * s + 1:2 * s + 2],
                                             e2[:], ow[:, 2 * s:2 * s + 1])
                        nc.vector.tensor_copy(oi[:, 2 * s:2 * s + 2],
                                              i8[:, 0:2].bitcast(I32))

                    t0 = c * CHUNK
                    nc.sync.dma_start(
                        out=ow_d[t0:t0 + CHUNK, :].rearrange(
                            "(s p) j -> p s j", s=TSUB),
                        in_=ow[:])
                    nc.sync.dma_start(
                        out=oi_d[t0:t0 + CHUNK, :].rearrange(
                            "(s p) j -> p s j", s=TSUB),
                        in_=oi[:])

            finalize()

    nc.compile()
    return nc


_NC = None


def _get_nc():
    global _NC
    if _NC is None:
        _NC = _build()
    return _NC


def run(h, weight, bias, trace=False):
    nc = _get_nc()
    h = np.ascontiguousarray(h, dtype=np.float32)
    weight = np.ascontiguousarray(weight, dtype=np.float32)
    bias = np.ascontiguousarray(bias, dtype=np.float32)
    in_maps = [{"h": h[i * B_SHARD:(i + 1) * B_SHARD], "weight": weight,
                "bias": bias} for i in range(N_CORES)]
    res = run_bass_kernel_spmd(nc, in_maps, list(range(N_CORES)), trace=trace)
    tw = np.concatenate([res.results[i]["topk_w"] for i in range(N_CORES)], 0)
    ti = np.concatenate([res.results[i]["topk_idx"] for i in range(N_CORES)], 0)
    return (tw.astype(np.float32), ti.astype(np.int32)), res


def kernel(h, weight, bias):
    (tw, ti), _ = run(h, weight, bias)
    return tw, ti
